# revision 14
# baseline (speedup 1.0000x reference)
"""BiLSTM tagger on 8 Trainium2 NeuronCores.

Reference computation (S=512, B=64, V=100000, E=128, H=256, T=64):
    x  = emb[inputs]                                  # [S,B,E]
    hf = LSTM_f(x);  hb = reverse(LSTM_b(reverse(x))) # [S,B,H] each
    out = concat(hf,hb) @ W_out.T + b_out             # [S,B,T]

Sharding: data-parallel over batch *and* direction.  Cores 0-3 run the
forward LSTM on batch slices of 16; cores 4-7 run the backward LSTM
(time-reversed indices) on the same batch slices.  Each core computes a
partial output projection with its direction's half of W_out; the host
sums fwd+bwd partials and adds b_out.

Per-core device pipeline (all compute on the NeuronCore):
  1. indirect-DMA gather of embedding rows (bf16 table) -> [tok,E] tiles
  2. PE transpose -> xT [E, 8192]
  3. x-projection GEMM (W_ih, bf16) + bias -> xpT [128, S*128] bf16 in SBUF
     (gate rows permuted to chunk order [i0,i1,f0,f1,o0,o1,g0,g1])
  4. 512-step LSTM scan: per step 16 matmuls (W_hh stationary, bf16,
     fast-weight-load) into PSUM, gates on scalar/vector engines in a
     [128, 8*16] packed layout, fp32 cell state, bf16 h
  5. output projection GEMM from saved h history, partials DMA'd out
"""

import sys

for _p in ("/opt/trn_rl_repo",):
    if _p not in sys.path:
        sys.path.insert(0, _p)

import numpy as np
import ml_dtypes

import concourse.bass as bass
import concourse.bacc as bacc
import concourse.mybir as mybir
import concourse.tile as tile
from concourse.bass import ts
from concourse.bass_utils import run_bass_kernel_spmd
from concourse.masks import make_identity

BF16 = mybir.dt.bfloat16
F32 = mybir.dt.float32
AF = mybir.ActivationFunctionType

S, B, V, E, H, T = 512, 64, 100000, 128, 256, 64
NCORES = 8
BL = B // (NCORES // 2)      # 16 batch per core
NTOK = S * BL                # 8192 tokens per core
G4H = 4 * H                  # 1024 gate rows
NCH = G4H // 128             # 8 gate-row chunks
NJT = NTOK // 128            # 64 gather tiles
NSL = NTOK // 512            # 16 GEMM slices

# gate-row permutation: torch order i,f,g,o -> chunk order i,f,o,g so the
# sigmoid gates (i,f,o) are contiguous in the packed layout
_PERM = np.concatenate(
    [np.arange(0, 2 * H), np.arange(3 * H, 4 * H), np.arange(2 * H, 3 * H)]
)


def build_program(n_steps: int = S) -> bass.Bass:
    NTOK = n_steps * BL
    NJT = NTOK // 128
    NSL = NTOK // 512

    nc = bacc.Bacc("TRN2", target_bir_lowering=False, debug=False)

    emb_d = nc.declare_dram_parameter("emb", [V, E], BF16, isOutput=False)
    idx_d = nc.declare_dram_parameter("idx", [128, NJT], mybir.dt.int32, isOutput=False)
    wih_d = nc.declare_dram_parameter("wih", [128, G4H], BF16, isOutput=False)
    whh_d = nc.declare_dram_parameter("whh", [H, G4H], BF16, isOutput=False)
    bias_d = nc.declare_dram_parameter("bias", [128, NCH], F32, isOutput=False)
    wout_d = nc.declare_dram_parameter("wout", [H, T], BF16, isOutput=False)
    out_d = nc.declare_dram_parameter("out", [T, NTOK], F32, isOutput=True)

    with tile.TileContext(nc) as tc:
        with (
            tc.tile_pool(name="persist", bufs=1) as pp,
            tc.tile_pool(name="tpsum", bufs=2, space="PSUM") as tpp,
            tc.tile_pool(name="gpsum", bufs=2, space="PSUM") as gpp,
            tc.tile_pool(name="spsum", bufs=2, space="PSUM") as spp,
            tc.tile_pool(name="opsum", bufs=2, space="PSUM") as opp,
            tc.tile_pool(name="gwork", bufs=3) as gwp,
            tc.tile_pool(name="swork", bufs=3) as swp,
        ):
            # ---- persistent SBUF tensors ----
            idx_sb = pp.tile([128, NJT], mybir.dt.int32, tag="idx")
            wih_sb = pp.tile([128, G4H], BF16, tag="wih")
            whh0_sb = pp.tile([128, G4H], BF16, tag="whh0")
            whh1_sb = pp.tile([128, G4H], BF16, tag="whh1")
            bias_sb = pp.tile([128, NCH], F32, tag="bias")
            wout_sb = pp.tile([128, 2 * T], BF16, tag="wout")
            ident = pp.tile([128, 128], BF16, tag="ident")
            xpT = pp.tile([128, n_steps * 128], BF16, tag="xpT")
            hs0 = pp.tile([128, (n_steps + 1) * BL], BF16, tag="hs0")
            hs1 = pp.tile([128, (n_steps + 1) * BL], BF16, tag="hs1")
            cst = pp.tile([128, 2 * BL], F32, tag="cst")
            # flat gather buffer: every gather writes a virgin region (the
            # dynamic-DMA descriptor has a single sem-wait slot, so a gather
            # may carry at most one dependency).  Each [tok,E] region is then
            # transposed in place (PE transpose -> PSUM -> DVE copy back) to
            # [E,tok], and the GEMM reads the buffer directly as rhs.
            xgb = pp.tile([128, NTOK], BF16, tag="xgb")

            # ---- load constants ----
            # idx goes through gpsimd's SWDGE queue (same queue as the
            # indirect gathers) so the gather needs no cross-queue wait:
            # the dynamic-DMA descriptor format only fits one sem wait.
            nc.gpsimd.dma_start(out=idx_sb[:], in_=idx_d[:])
            nc.sync.dma_start(out=wih_sb[:], in_=wih_d[:])
            nc.sync.dma_start(out=whh0_sb[:], in_=whh_d[0:128, :])
            nc.sync.dma_start(out=whh1_sb[:], in_=whh_d[128:256, :])
            nc.sync.dma_start(out=bias_sb[:], in_=bias_d[:])
            nc.sync.dma_start(out=wout_sb[:, 0:T], in_=wout_d[0:128, :])
            nc.sync.dma_start(out=wout_sb[:, T : 2 * T], in_=wout_d[128:256, :])
            make_identity(nc, ident[:])

            nc.gpsimd.memset(hs0[:, 0:BL], 0.0)
            nc.gpsimd.memset(hs1[:, 0:BL], 0.0)
            nc.gpsimd.memset(cst[:], 0.0)

            # ---- gather + in-place transpose + x-projection GEMM ----
            xp4 = xpT[:].rearrange("p (t c b) -> p t c b", c=NCH, b=BL)
            for j in range(NJT):
                nc.gpsimd.indirect_dma_start(
                    out=xgb[:, ts(j, 128)],
                    out_offset=None,
                    in_=emb_d[:],
                    in_offset=bass.IndirectOffsetOnAxis(
                        ap=idx_sb[:, j : j + 1], axis=0
                    ),
                )
                tp = tpp.tile([128, 128], BF16, tag="tp")
                nc.tensor.transpose(tp[:], xgb[:, ts(j, 128)], ident[:])
                # scalar-engine copy: the DVE copy lowers to the one-wait-slot
                # S4D4_TR encoding, and this instruction needs two waits
                # (PSUM ready + gather WAW on the region)
                nc.scalar.copy(out=xgb[:, ts(j, 128)], in_=tp[:])
            for s in range(NSL):
                for c in range(NCH):
                    pg = gpp.tile([128, 512], F32, tag="pg")
                    nc.tensor.matmul(
                        pg[:],
                        lhsT=wih_sb[:, ts(c, 128)],
                        rhs=xgb[:, ts(s, 512)],
                        start=True,
                        stop=True,
                    )
                    # copy+bias into the packed [t, c, b] layout
                    nc.scalar.activation(
                        out=xp4[:, s * 32 : (s + 1) * 32, c, :],
                        in_=pg[:].rearrange("p (t b) -> p t b", b=BL),
                        func=AF.Identity,
                        bias=bias_sb[:, c : c + 1],
                        scale=1.0,
                    )

            # ---- LSTM scan ----
            for t in range(n_steps):
                ps = spp.tile([128, 128], F32, tag="ps")
                h0 = hs0[:, ts(t, BL)]
                h1 = hs1[:, ts(t, BL)]
                for c in range(NCH):
                    nc.tensor.matmul(
                        ps[:, ts(c, BL)],
                        lhsT=whh0_sb[:, ts(c, 128)],
                        rhs=h0,
                        start=True,
                        stop=False,
                    )
                    nc.tensor.matmul(
                        ps[:, ts(c, BL)],
                        lhsT=whh1_sb[:, ts(c, 128)],
                        rhs=h1,
                        start=False,
                        stop=True,
                    )
                gp = gwp.tile([128, 128], F32, tag="gp")
                nc.vector.tensor_add(gp[:], ps[:], xpT[:, ts(t, 128)])
                gs = gwp.tile([128, 128], F32, tag="gs")
                nc.scalar.activation(gs[:, 0:96], gp[:, 0:96], AF.Sigmoid)
                nc.scalar.activation(gs[:, 96:128], gp[:, 96:128], AF.Tanh)
                t1 = swp.tile([128, 2 * BL], F32, tag="t1")
                t2 = swp.tile([128, 2 * BL], F32, tag="t2")
                nc.vector.tensor_mul(t1[:], gs[:, 32:64], cst[:])      # f * c
                nc.vector.tensor_mul(t2[:], gs[:, 0:32], gs[:, 96:128])  # i * g
                nc.vector.tensor_add(cst[:], t1[:], t2[:])
                th = swp.tile([128, 2 * BL], F32, tag="th")
                nc.scalar.activation(th[:], cst[:], AF.Tanh)
                nc.vector.tensor_mul(
                    hs0[:, ts(t + 1, BL)], gs[:, 64:80], th[:, 0:BL]
                )
                nc.vector.tensor_mul(
                    hs1[:, ts(t + 1, BL)], gs[:, 80:96], th[:, BL : 2 * BL]
                )

            # ---- output projection ----
            for s in range(NSL):
                po = opp.tile([T, 512], F32, tag="po")
                nc.tensor.matmul(
                    po[:],
                    lhsT=wout_sb[:, 0:T],
                    rhs=hs0[:, BL + s * 512 : BL + (s + 1) * 512],
                    start=True,
                    stop=False,
                )
                nc.tensor.matmul(
                    po[:],
                    lhsT=wout_sb[:, T : 2 * T],
                    rhs=hs1[:, BL + s * 512 : BL + (s + 1) * 512],
                    start=False,
                    stop=True,
                )
                og = swp.tile([T, 512], F32, tag="og")
                nc.vector.tensor_copy(out=og[:], in_=po[:])
                nc.sync.dma_start(out=out_d[:, ts(s, 512)], in_=og[:])

    nc.compile()
    return nc


_PROGRAM_CACHE: list = []


def _get_program() -> bass.Bass:
    if not _PROGRAM_CACHE:
        _PROGRAM_CACHE.append(build_program())
    return _PROGRAM_CACHE[0]


def _core_inputs(core, inputs_i32, emb_bf, weights):
    fwd = core < 4
    bsl = slice((core % 4) * BL, (core % 4) * BL + BL)
    W_ih, W_hh, b_ih, b_hh, W_out = weights[0 if fwd else 1]

    ids = inputs_i32[:, bsl]
    if not fwd:
        ids = ids[::-1]
    idx_t = np.ascontiguousarray(ids.reshape(NJT, 128).T).astype(np.int32)

    Wihp = W_ih[_PERM]                       # [4H, E]
    wih = np.ascontiguousarray(Wihp.T).astype(ml_dtypes.bfloat16)  # [E, 4H]
    Whhp = W_hh[_PERM]                       # [4H, H]
    whh = np.ascontiguousarray(Whhp.T).astype(ml_dtypes.bfloat16)  # [H, 4H]
    bp = (b_ih + b_hh)[_PERM].astype(np.float32)
    bias = np.ascontiguousarray(bp.reshape(NCH, 128).T)            # [128, 8]
    wo = W_out[:, 0:H] if fwd else W_out[:, H : 2 * H]             # [T, H]
    wout = np.ascontiguousarray(wo.T).astype(ml_dtypes.bfloat16)   # [H, T]

    return {
        "emb": emb_bf,
        "idx": idx_t,
        "wih": wih,
        "whh": whh,
        "bias": bias,
        "wout": wout,
    }


def kernel(**inputs) -> np.ndarray:
    ids = np.asarray(inputs["inputs"]).astype(np.int32)      # [S, B]
    emb_bf = np.asarray(inputs["emb"], np.float32).astype(ml_dtypes.bfloat16)
    weights = [
        (
            np.asarray(inputs["W_ih_f"], np.float32),
            np.asarray(inputs["W_hh_f"], np.float32),
            np.asarray(inputs["b_ih_f"], np.float32),
            np.asarray(inputs["b_hh_f"], np.float32),
            np.asarray(inputs["W_out"], np.float32),
        ),
        (
            np.asarray(inputs["W_ih_b"], np.float32),
            np.asarray(inputs["W_hh_b"], np.float32),
            np.asarray(inputs["b_ih_b"], np.float32),
            np.asarray(inputs["b_hh_b"], np.float32),
            np.asarray(inputs["W_out"], np.float32),
        ),
    ]

    nc = _get_program()
    in_maps = [_core_inputs(k, ids, emb_bf, weights) for k in range(NCORES)]
    import os

    kw = {}
    if os.environ.get("KERNEL_TRACE"):
        kw = {"trace": True, "tmpdir": os.environ.get("KERNEL_TRACE_DIR") or None}
    r = run_bass_kernel_spmd(nc, in_maps, list(range(NCORES)), **kw)
    global LAST_RESULTS
    LAST_RESULTS = r
    res = r.results

    out = np.zeros((S, B, T), np.float32)
    for core in range(NCORES):
        bsl = slice((core % 4) * BL, (core % 4) * BL + BL)
        part = res[core]["out"]                  # [T, NTOK], tokens t-major
        part = part.T.reshape(S, BL, T)
        if core >= 4:
            part = part[::-1]
        out[:, bsl, :] += part
    out += np.asarray(inputs["b_out"], np.float32)
    return out
